# revision 10
# baseline (speedup 1.0000x reference)
"""Trainium2 Bass kernel for grouped block-diagonal MLP (gnn_message_passing).

Computation: out[b, 3g+j] = sum_i x[b, 15g+i] * W[g, j, i]   (g<25, i<15, j<3)
Equivalent to out = x @ Wd where Wd is a [375, 75] block-diagonal matrix built
from the 25 stacked [3, 15] Linear weights (scattered per k_idx/v_idx).

Strategy (pure data parallel, 8 cores):
  - host: transpose x to xT [375, B] and cast to bf16 (halves HBM reads and
    puts the contraction dim on SBUF partitions with zero device transposes),
    shard batch dim across 8 cores, replicate dense Wd (bf16).
  - per core: stream xT in [128, NBLK] chunk tiles (16 KB/partition DMAs on
    the sync HWDGE ring). Matmuls run c-outer: one Wd chunk stays stationary
    across a group of 7 x 512-col sub-blocks (7 PSUM banks), so LDWEIGHTS is
    amortized and matmuls pipeline back-to-back; the 3 chunk passes
    accumulate into the same PSUM banks (interleaved accumulation groups,
    skip_group_check). PSUM->SBUF bf16 casts alternate Vector/Scalar; output
    DMAs go out on the scalar HWDGE ring to avoid head-of-line blocking.
  - host: concat, transpose back, cast to f32.
"""

import numpy as np
import ml_dtypes

BF16 = ml_dtypes.bfloat16

B = 262144
NCORES = 8
B_CORE = B // NCORES  # 32768
F = 375   # input cols  (25 groups * 15)
O = 75    # output cols (25 groups * 3)
OUT_DIM = 75
CHUNKS = [(0, 128), (128, 128), (256, 119)]  # (offset, size) along F
NBLK = 4096                  # columns (batch rows) per streamed DMA block
NB = B_CORE // NBLK          # input DMA blocks
OBLK = 8192                  # columns per output DMA block
NSUB_TOT = B_CORE // 512     # 64 512-col matmul sub-blocks
GROUP = 4                    # sub-blocks per PSUM-resident weight pass

_compiled = {}


def _build_bass():
    import concourse.mybir as mybir
    import concourse.tile as tile
    from concourse import bacc

    f32 = mybir.dt.float32
    bf16 = mybir.dt.bfloat16
    nc = bacc.Bacc()
    x_d = nc.dram_tensor("xt", [F, B_CORE], bf16, kind="ExternalInput")
    w_d = nc.dram_tensor("wd", [128, 3, 128], bf16, kind="ExternalInput")
    o_d = nc.dram_tensor("out", [O, B_CORE], bf16, kind="ExternalOutput")

    groups = [
        list(range(g0, min(NSUB_TOT, g0 + GROUP)))
        for g0 in range(0, NSUB_TOT, GROUP)
    ]

    with tile.TileContext(nc) as tc:
        with (
            tc.tile_pool(name="const", bufs=1) as cpool,
            tc.tile_pool(name="xin", bufs=5) as xpool,
            tc.tile_pool(name="res", bufs=3) as rpool,
            tc.tile_pool(name="acc", bufs=8, space="PSUM") as pacc,
        ):
            wd = cpool.tile([128, 3, 128], bf16)
            nc.sync.dma_start(wd[:], w_d[:])

            # Absorb the wd DMA dependency into PE once, so steady-state
            # matmuls only wait on their x DMA / PSUM-free sems. The warm
            # tile cycles through the same 8-bank acc pool.
            warm = pacc.tile([128, 128], f32, name="acc", tag="acc")
            for c in range(3):
                nc.tensor.matmul(
                    warm[:],
                    wd[:, c, :],
                    wd[:, 0, :],
                    start=(c == 0),
                    stop=(c == 2),
                )

            xt = {}       # blk -> [chunk tiles]
            stage = {}    # out-blk -> staging tile
            loaded = -1

            def ensure_block(blk):
                nonlocal loaded
                while loaded < blk:
                    loaded += 1
                    b0 = loaded * NBLK
                    tiles = []
                    for c, (off, sz) in enumerate(CHUNKS):
                        t = xpool.tile([128, NBLK], bf16, tag=f"x{c}")
                        nc.sync.dma_start(
                            t[:sz, :], x_d[off : off + sz, b0 : b0 + NBLK]
                        )
                        tiles.append(t)
                    xt[loaded] = tiles

            for gi, g in enumerate(groups):
                ensure_block((g[-1] * 512) // NBLK)
                accs = {}
                for c, (off, sz) in enumerate(CHUNKS):
                    for s in g:
                        blk = (s * 512) // NBLK
                        col = s * 512 - blk * NBLK
                        if c == 0:
                            accs[s] = pacc.tile([128, 512], f32, name="acc", tag="acc")
                        nc.tensor.matmul(
                            accs[s][:],
                            wd[:sz, c, :],
                            xt[blk][c][:sz, col : col + 512],
                            start=(c == 0),
                            stop=(c == 2),
                            skip_group_check=True,
                        )
                for i, s in enumerate(g):
                    oblk = (s * 512) // OBLK
                    ocol = s * 512 - oblk * OBLK
                    if oblk not in stage:
                        stage[oblk] = rpool.tile([O, OBLK], bf16, name="stage", tag="stage")
                    if s % 2 == 0:
                        nc.vector.tensor_copy(
                            stage[oblk][:, ocol : ocol + 512], accs[s][:O, :]
                        )
                    else:
                        nc.scalar.copy(
                            stage[oblk][:, ocol : ocol + 512], accs[s][:O, :]
                        )
                    if s % (OBLK // 512) == (OBLK // 512) - 1:
                        # last sub of this output block: ship it (ACT ring)
                        nc.scalar.dma_start(
                            o_d[:, oblk * OBLK : (oblk + 1) * OBLK],
                            stage[oblk][:],
                        )
    nc.compile()
    return nc


def _get_nc():
    if "nc" not in _compiled:
        _compiled["nc"] = _build_bass()
    return _compiled["nc"]


def _build_wd(W, k_idx, v_idx):
    """Dense [128, 3, 128] chunked block-diagonal weight (bf16) from stacked W."""
    Wd = np.zeros((384, 128), dtype=np.float32)
    kk = np.asarray(k_idx)
    vv = np.asarray(v_idx)
    Ww = np.asarray(W, dtype=np.float32)
    # Wd[k_idx[g,i], v_idx[g,j]] = W[g, j, i]
    Wd[kk[:, :, None], vv[:, None, :]] = Ww.transpose(0, 2, 1)
    # [384, 128] -> [3, 128, 128] -> [128, 3, 128]; cols 75..127 are zero
    # padding so NumWeights==128 enables FWL (background weight-buffer loads).
    return np.ascontiguousarray(
        Wd.reshape(3, 128, 128).transpose(1, 0, 2).astype(BF16)
    )


def kernel(x, W, k_idx, v_idx, **_unused):
    from concourse.bass_utils import run_bass_kernel_spmd

    x = np.asarray(x)
    wd = _build_wd(W, k_idx, v_idx)
    nc = _get_nc()

    xb = x.astype(BF16)  # one pass f32->bf16, then per-shard transpose
    in_maps = [
        {
            "xt": np.ascontiguousarray(xb[i * B_CORE : (i + 1) * B_CORE].T),
            "wd": wd,
        }
        for i in range(NCORES)
    ]
    res = run_bass_kernel_spmd(nc, in_maps, list(range(NCORES)))
    parts = [res.results[i]["out"] for i in range(NCORES)]
    got = np.concatenate(parts, axis=1)  # [75, B] bf16
    out_full = np.ascontiguousarray(got.T).astype(np.float32)  # [B, 75]

    vflat = np.asarray(v_idx).reshape(-1)
    if vflat.shape[0] == OUT_DIM and np.array_equal(vflat, np.arange(OUT_DIM)):
        return out_full
    out = np.zeros((x.shape[0], OUT_DIM), dtype=np.float32)
    out[:, vflat] = out_full
    return out
